# revision 34
# baseline (speedup 1.0000x reference)
"""Trainium2 Bass kernel for nn_BERTEmbedding_65274912964883.

out[b, l, :] = token_table[seq[b, l]]
             + mean_{g in genres(seq[b, l])} genre_table[g]
             + pos_table[l]

Strategy (8 NeuronCores, SPMD, no collectives):
  - The genre mean depends only on the token id, so the host folds it into
    the token table once: ftab[v] = token_table[v] + genre_mean[v].
  - Each core only touches its own 6400 tokens, so the host also builds a
    per-core COMPACT table ctab = ftab[unique(seq_core)] (f16, <=6400 rows)
    and remaps the sequence to compact ids -- which fit dma_gather's int16
    index constraint with room to spare.
  - The gather itself is chunked nc.gpsimd.dma_gather ops (bass's custom
    SWDGE Q7 path): ONE 256B descriptor per token. Gathered token i lands
    on partition i%128, subtile i//128 -- exactly the natural device
    layout, so positions stay aligned. The Q7's descriptor-emission rate
    (~8.5ns/descriptor measured on HW, for walrus indirect DMA and
    dma_gather alike) paces the kernel at ~55us/core; every other engine
    rides in its shadow.
  - positional rows come from a host-prebuilt rotated table (28 rotations,
    f16) resident in SBUF; chunk boundaries are chosen so each chunk's
    positional block is one contiguous slice -> ONE DVE add per chunk
    (f16+f16 at 2x DVE rate) fused with the f16 downcast.
  - Chunks are triple-buffered: gathers run up to 3 chunks ahead of the
    DVE add + output DMA chain. The tapered tail shortens the drain.
    Chunks stay <= 7 subtiles: 56 descriptors/engine fits the SWDGE
    descriptor ring without mid-op reclaim (bigger ops crash or race on
    real HW; enlarging dynamic_dma_scratch_size does not help).
  - Device writes output partition-major [128, NSUB, D] f16; host
    un-permutes and upcasts.
"""

import numpy as np

import concourse.bacc as bacc
import concourse.mybir as mybir
import concourse.tile as tile
from concourse.bass_utils import run_bass_kernel_spmd

VOCAB = 100000
D = 128
G = 21          # genre ids are in [0, 20]
MAXG = 8
B, L = 256, 200
NCORES = 8
BC = B // NCORES          # sequences per core
N = BC * L                # tokens per core (6400)
SUB = 128                 # tokens per subtile (partition dim)
NSUB = N // SUB           # 50
NROT = 25                 # distinct values of (128*i) % 200
NROTX = 28                # extended with 3 duplicates so chunks never wrap
NIDXCOL = N // 16         # idx columns (wrapped in 16 partitions)
# chunk starts i0 must satisfy (i0 % NROT) + ck <= NROTX so each chunk's
# positional block is contiguous in the rotated table; chunks are capped
# at 7 subtiles (56 descriptors per SDMA engine) so each dma_gather fits
# the Q7's per-engine descriptor ring without mid-op reclaim -- larger
# ops crash (single_packet) or race (multi-packet) on real HW
CHUNKS = [7, 7, 7, 7, 7, 7, 4, 4]

F32 = mybir.dt.float32
F16 = mybir.dt.float16
I16 = mybir.dt.int16

assert sum(CHUNKS) == NSUB
_i0 = 0
for _ck in CHUNKS:
    assert (_i0 % NROT) + _ck <= NROTX, (_i0, _ck)
    _i0 += _ck


def emit_core_kernel(tc, idx, ctab, posfull, out):
    """Emit the per-core kernel into TileContext `tc`.

    idx    : DRAM [128, NIDXCOL] int16 compact token ids, wrapped layout:
             token n at (n % 16, n // 16), replicated x8 down partitions
    ctab   : DRAM [N, D] f16 per-core compact fused table
    posfull: DRAM [128, N] f16 positional table, feature-major
             (posfull[d, n] = pos_table[n % L, d])
    out    : DRAM [128, N] f16 feature-major, out[d, n] = embedding of
             token n, feature d
    """
    nc = tc.nc

    with (
        tc.tile_pool(name="const", bufs=1) as cpool,
        tc.tile_pool(name="work", bufs=2) as wpool,
    ):
        # idx first: the first gather depends only on its first slice
        idx_sb = cpool.tile([128, NIDXCOL], I16)
        c0 = CHUNKS[0] * 8
        nc.sync.dma_start(out=idx_sb[:, 0:c0], in_=idx[:, 0:c0])
        nc.sync.dma_start(out=idx_sb[:, c0:NIDXCOL], in_=idx[:, c0:NIDXCOL])
        posfull_sb = cpool.tile([128, N], F16)
        nc.sync.dma_start(out=posfull_sb[:], in_=posfull)

        i0 = 0
        for ck in CHUNKS:
            # one batched TRANSPOSE gather: token n's row is scattered
            # feature-major (out[d, n]) via the xbar; one 256B desc/token
            g_sb = wpool.tile([128, ck * SUB], F16, tag="g", bufs=3)
            nc.gpsimd.dma_gather(
                out_ap=g_sb[:].rearrange("p (c t) -> p c t", c=1),
                in_ap=ctab,
                idxs_ap=idx_sb[:, i0 * 8:(i0 + ck) * 8],
                num_idxs=ck * SUB,
                num_idxs_reg=ck * SUB,
                elem_size=D,
                transpose=True,
            )
            # fused positional add, one DVE op per chunk
            o_sb = wpool.tile([128, ck * SUB], F16, tag="o", bufs=3)
            nc.vector.tensor_tensor(
                out=o_sb[:],
                in0=g_sb[:],
                in1=posfull_sb[:, i0 * SUB:(i0 + ck) * SUB],
                op=mybir.AluOpType.add,
            )
            nc.sync.dma_start(
                out=out[:, i0 * SUB:(i0 + ck) * SUB],
                in_=o_sb[:],
            )
            i0 += ck


def build_nc():
    nc = bacc.Bacc("TRN2", target_bir_lowering=False, debug=False)
    idx = nc.dram_tensor("idx", [128, NIDXCOL], I16, kind="ExternalInput").ap()
    ctab = nc.dram_tensor("ctab", [N, D], F16, kind="ExternalInput").ap()
    posfull = nc.dram_tensor(
        "posfull", [128, N], F16, kind="ExternalInput").ap()
    out = nc.dram_tensor("out", [128, N], F16, kind="ExternalOutput").ap()

    with tile.TileContext(nc) as tc:
        emit_core_kernel(tc, idx, ctab, posfull, out)
    nc.compile()
    return nc


_NC_CACHE = None


def _get_nc():
    global _NC_CACHE
    if _NC_CACHE is None:
        _NC_CACHE = build_nc()
    return _NC_CACHE


def make_ftab(token_table, genre_table, token_genre_ids, genre_counts):
    """fused table: ftab[v] = token_table[v] + mean genre row, f32."""
    gids = np.asarray(token_genre_ids).astype(np.int64)        # [V, MAXG]
    cnt = np.asarray(genre_counts).astype(np.int64)            # [V]
    mask = np.arange(MAXG)[None, :] < cnt[:, None]             # [V, MAXG]
    flat = (np.arange(VOCAB, dtype=np.int64)[:, None] * G + gids)[mask]
    W = np.bincount(flat, minlength=VOCAB * G).reshape(VOCAB, G)
    gmean = (W.astype(np.float32) @ np.asarray(genre_table, dtype=np.float32))
    gmean /= cnt[:, None].astype(np.float32)
    return np.asarray(token_table, dtype=np.float32) + gmean


def make_posfull(pos_table):
    posT = np.asarray(pos_table, dtype=np.float32).T      # [D, L]
    return np.ascontiguousarray(
        np.tile(posT, (1, N // L))).astype(np.float16)    # [D, N]


def prep_host_inputs(sequence, token_table, genre_table, pos_table,
                     token_genre_ids, genre_counts):
    """Host-side sharding / layout prep. Returns in_maps for the 8 cores."""
    seq = np.ascontiguousarray(
        np.asarray(sequence).astype(np.int64)).reshape(B, L)
    ftab = make_ftab(token_table, genre_table, token_genre_ids, genre_counts)
    posfull = make_posfull(pos_table)

    in_maps = []
    for c in range(NCORES):
        seq_core = seq[c * BC:(c + 1) * BC].reshape(N)
        uniq, inv = np.unique(seq_core, return_inverse=True)
        ctab = np.zeros((N, D), dtype=np.float16)
        ctab[:len(uniq)] = ftab[uniq].astype(np.float16)
        # wrapped idx layout: token n at (n % 16, n // 16), replicated x8
        idx16 = np.ascontiguousarray(
            inv.astype(np.int16).reshape(NIDXCOL, 16).T)
        idx = np.tile(idx16, (8, 1))
        in_maps.append({
            "idx": idx,
            "ctab": ctab,
            "posfull": posfull,
        })
    return in_maps


def postprocess(results):
    """Un-permute per-core outputs and concatenate to [B, L, D] f32."""
    outs = []
    for c in range(NCORES):
        o = results[c]["out"].astype(np.float32)  # [128, N] feature-major
        outs.append(np.ascontiguousarray(o.T).reshape(BC, L, D))
    return np.concatenate(outs, axis=0)


def kernel(sequence, token_table, genre_table, pos_table, token_genre_ids,
           genre_counts):
    nc = _get_nc()
    in_maps = prep_host_inputs(sequence, token_table, genre_table, pos_table,
                               token_genre_ids, genre_counts)
    res = run_bass_kernel_spmd(nc, in_maps, core_ids=list(range(NCORES)))
    return postprocess(res.results)


# revision 35
# speedup vs baseline: 1.0289x; 1.0289x over previous
"""Trainium2 Bass kernel for nn_BERTEmbedding_65274912964883.

out[b, l, :] = token_table[seq[b, l]]
             + mean_{g in genres(seq[b, l])} genre_table[g]
             + pos_table[l]

Strategy (8 NeuronCores, SPMD, no collectives):
  - The genre mean depends only on the token id, so the host folds it into
    the token table once: ftab[v] = token_table[v] + genre_mean[v].
  - Each core only touches its own 6400 tokens, so the host also builds a
    per-core COMPACT table ctab = ftab[unique(seq_core)] (f16, <=6400 rows)
    and remaps the sequence to compact ids -- which fit dma_gather's int16
    index constraint with room to spare.
  - The gather itself is chunked nc.gpsimd.dma_gather ops (bass's custom
    SWDGE Q7 path): ONE 256B descriptor per token. Gathered token i lands
    on partition i%128, subtile i//128 -- exactly the natural device
    layout, so positions stay aligned. The Q7's descriptor-emission rate
    (~8.5ns/descriptor measured on HW, for walrus indirect DMA and
    dma_gather alike) paces the kernel at ~55us/core; every other engine
    rides in its shadow.
  - positional rows come from a host-prebuilt rotated table (28 rotations,
    f16) resident in SBUF; chunk boundaries are chosen so each chunk's
    positional block is one contiguous slice -> ONE DVE add per chunk
    (f16+f16 at 2x DVE rate) fused with the f16 downcast.
  - Chunks are triple-buffered: gathers run up to 3 chunks ahead of the
    DVE add + output DMA chain. The tapered tail shortens the drain.
    Chunks stay <= 7 subtiles: 56 descriptors/engine fits the SWDGE
    descriptor ring without mid-op reclaim (bigger ops crash or race on
    real HW; enlarging dynamic_dma_scratch_size does not help).
  - Device writes output partition-major [128, NSUB, D] f16; host
    un-permutes and upcasts.
"""

import numpy as np

import concourse.bacc as bacc
import concourse.mybir as mybir
import concourse.tile as tile
from concourse.bass_utils import run_bass_kernel_spmd

VOCAB = 100000
D = 128
G = 21          # genre ids are in [0, 20]
MAXG = 8
B, L = 256, 200
NCORES = 8
BC = B // NCORES          # sequences per core
N = BC * L                # tokens per core (6400)
SUB = 128                 # tokens per subtile (partition dim)
NSUB = N // SUB           # 50
NROT = 25                 # distinct values of (128*i) % 200
NROTX = 28                # extended with 3 duplicates so chunks never wrap
NIDXCOL = N // 16         # idx columns (wrapped in 16 partitions)
# chunk starts i0 must satisfy (i0 % NROT) + ck <= NROTX so each chunk's
# positional block is contiguous in the rotated table; chunks are capped
# at 7 subtiles (56 descriptors per SDMA engine) so each dma_gather fits
# the Q7's per-engine descriptor ring without mid-op reclaim -- larger
# ops crash (single_packet) or race (multi-packet) on real HW
CHUNKS = [7, 7, 7, 7, 7, 7, 4, 4]

F32 = mybir.dt.float32
F16 = mybir.dt.float16
I16 = mybir.dt.int16

assert sum(CHUNKS) == NSUB
_i0 = 0
for _ck in CHUNKS:
    assert (_i0 % NROT) + _ck <= NROTX, (_i0, _ck)
    _i0 += _ck


def emit_core_kernel(tc, idx, ctab, posrot, out):
    """Emit the per-core kernel into TileContext `tc`.

    idx    : DRAM [128, NIDXCOL] int16 compact token ids, wrapped layout:
             token n at (n % 16, n // 16), replicated x8 down partitions
    ctab   : DRAM [N, D] f16 per-core compact fused table
    posrot : DRAM [128, NROTX*D] f16 rotated positional table
    out    : DRAM [128, NSUB, D] f16, out[p, i, :] = embedding of token i*128+p
    """
    nc = tc.nc

    with (
        tc.tile_pool(name="const", bufs=1) as cpool,
        tc.tile_pool(name="work", bufs=2) as wpool,
    ):
        # idx first: the first gather depends only on its first slice
        idx_sb = cpool.tile([128, NIDXCOL], I16)
        c0 = CHUNKS[0] * 8
        nc.sync.dma_start(out=idx_sb[:, 0:c0], in_=idx[:, 0:c0])
        nc.sync.dma_start(out=idx_sb[:, c0:NIDXCOL], in_=idx[:, c0:NIDXCOL])
        posrot_sb = cpool.tile([128, NROTX * D], F16)
        nc.sync.dma_start(out=posrot_sb[:], in_=posrot)

        i0 = 0
        for ck in CHUNKS:
            # one batched gather: ck*128 tokens, one 256B descriptor each;
            # token i*128+p lands on partition p, local subtile i
            g_sb = wpool.tile([128, ck * D], F16, tag="g", bufs=3)
            nc.gpsimd.dma_gather(
                out_ap=g_sb[:].rearrange("p (j d) -> p j d", d=D),
                in_ap=ctab,
                idxs_ap=idx_sb[:, i0 * 8:(i0 + ck) * 8],
                num_idxs=ck * SUB,
                num_idxs_reg=ck * SUB,
                elem_size=D,
            )
            # fused positional add, one DVE op per chunk
            r0 = i0 % NROT
            o_sb = wpool.tile([128, ck * D], F16, tag="o", bufs=3)
            nc.vector.tensor_tensor(
                out=o_sb[:],
                in0=g_sb[:],
                in1=posrot_sb[:, r0 * D:(r0 + ck) * D],
                op=mybir.AluOpType.add,
            )
            nc.sync.dma_start(
                out=out[:, i0:i0 + ck, :],
                in_=o_sb[:].rearrange("p (j d) -> p j d", d=D),
            )
            i0 += ck


def build_nc():
    nc = bacc.Bacc("TRN2", target_bir_lowering=False, debug=False)
    idx = nc.dram_tensor("idx", [128, NIDXCOL], I16, kind="ExternalInput").ap()
    ctab = nc.dram_tensor("ctab", [N, D], F16, kind="ExternalInput").ap()
    posrot = nc.dram_tensor(
        "posrot", [128, NROTX * D], F16, kind="ExternalInput").ap()
    out = nc.dram_tensor("out", [128, NSUB, D], F16, kind="ExternalOutput").ap()

    with tile.TileContext(nc) as tc:
        emit_core_kernel(tc, idx, ctab, posrot, out)
    nc.compile()
    return nc


_NC_CACHE = None


def _get_nc():
    global _NC_CACHE
    if _NC_CACHE is None:
        _NC_CACHE = build_nc()
    return _NC_CACHE


def make_ftab(token_table, genre_table, token_genre_ids, genre_counts):
    """fused table: ftab[v] = token_table[v] + mean genre row, f32."""
    gids = np.asarray(token_genre_ids).astype(np.int64)        # [V, MAXG]
    cnt = np.asarray(genre_counts).astype(np.int64)            # [V]
    mask = np.arange(MAXG)[None, :] < cnt[:, None]             # [V, MAXG]
    flat = (np.arange(VOCAB, dtype=np.int64)[:, None] * G + gids)[mask]
    W = np.bincount(flat, minlength=VOCAB * G).reshape(VOCAB, G)
    gmean = (W.astype(np.float32) @ np.asarray(genre_table, dtype=np.float32))
    gmean /= cnt[:, None].astype(np.float32)
    return np.asarray(token_table, dtype=np.float32) + gmean


def make_posrot(pos_table):
    pos = np.asarray(pos_table, dtype=np.float32)
    pr = np.zeros((128, NROTX * D), dtype=np.float32)
    p = np.arange(128)
    for r in range(NROTX):
        pr[:, r * D:(r + 1) * D] = pos[(128 * r + p) % L, :]
    return pr.astype(np.float16)


def prep_host_inputs(sequence, token_table, genre_table, pos_table,
                     token_genre_ids, genre_counts):
    """Host-side sharding / layout prep. Returns in_maps for the 8 cores."""
    seq = np.ascontiguousarray(
        np.asarray(sequence).astype(np.int64)).reshape(B, L)
    ftab = make_ftab(token_table, genre_table, token_genre_ids, genre_counts)
    posrot = make_posrot(pos_table)

    in_maps = []
    for c in range(NCORES):
        seq_core = seq[c * BC:(c + 1) * BC].reshape(N)
        uniq, inv = np.unique(seq_core, return_inverse=True)
        ctab = np.zeros((N, D), dtype=np.float16)
        ctab[:len(uniq)] = ftab[uniq].astype(np.float16)
        # wrapped idx layout: token n at (n % 16, n // 16), replicated x8
        idx16 = np.ascontiguousarray(
            inv.astype(np.int16).reshape(NIDXCOL, 16).T)
        idx = np.tile(idx16, (8, 1))
        in_maps.append({
            "idx": idx,
            "ctab": ctab,
            "posrot": posrot,
        })
    return in_maps


def postprocess(results):
    """Un-permute per-core outputs and concatenate to [B, L, D] f32."""
    outs = []
    for c in range(NCORES):
        o = results[c]["out"].astype(np.float32)  # [128, NSUB, D]
        outs.append(
            np.ascontiguousarray(o.transpose(1, 0, 2)).reshape(BC, L, D))
    return np.concatenate(outs, axis=0)


def kernel(sequence, token_table, genre_table, pos_table, token_genre_ids,
           genre_counts):
    nc = _get_nc()
    in_maps = prep_host_inputs(sequence, token_table, genre_table, pos_table,
                               token_genre_ids, genre_counts)
    res = run_bass_kernel_spmd(nc, in_maps, core_ids=list(range(NCORES)))
    return postprocess(res.results)


# revision 36
# speedup vs baseline: 1.0314x; 1.0024x over previous
"""Trainium2 Bass kernel for nn_BERTEmbedding_65274912964883.

out[b, l, :] = token_table[seq[b, l]]
             + mean_{g in genres(seq[b, l])} genre_table[g]
             + pos_table[l]

Strategy (8 NeuronCores, SPMD, no collectives):
  - The genre mean depends only on the token id, so the host folds it into
    the token table once: ftab[v] = token_table[v] + genre_mean[v].
  - Each core only touches its own 6400 tokens, so the host also builds a
    per-core COMPACT table ctab = ftab[unique(seq_core)] (f16, <=6400 rows)
    and remaps the sequence to compact ids -- which fit dma_gather's int16
    index constraint with room to spare.
  - The gather itself is chunked nc.gpsimd.dma_gather ops (bass's custom
    SWDGE Q7 path): ONE 256B descriptor per token. Gathered token i lands
    on partition i%128, subtile i//128 -- exactly the natural device
    layout, so positions stay aligned. The Q7's descriptor-emission rate
    (~8.5ns/descriptor measured on HW, for walrus indirect DMA and
    dma_gather alike) paces the kernel at ~55us/core; every other engine
    rides in its shadow.
  - positional rows come from a host-prebuilt rotated table (28 rotations,
    f16) resident in SBUF; chunk boundaries are chosen so each chunk's
    positional block is one contiguous slice -> ONE DVE add per chunk
    (f16+f16 at 2x DVE rate) fused with the f16 downcast.
  - Chunks are triple-buffered: gathers run up to 3 chunks ahead of the
    DVE add + output DMA chain. The tapered tail shortens the drain.
    Chunks stay <= 7 subtiles: 56 descriptors/engine fits the SWDGE
    descriptor ring without mid-op reclaim (bigger ops crash or race on
    real HW; enlarging dynamic_dma_scratch_size does not help).
  - Device writes output partition-major [128, NSUB, D] f16; host
    un-permutes and upcasts.
"""

import numpy as np

import concourse.bacc as bacc
import concourse.mybir as mybir
import concourse.tile as tile
from concourse.bass_utils import run_bass_kernel_spmd

VOCAB = 100000
D = 128
G = 21          # genre ids are in [0, 20]
MAXG = 8
B, L = 256, 200
NCORES = 8
BC = B // NCORES          # sequences per core
N = BC * L                # tokens per core (6400)
SUB = 128                 # tokens per subtile (partition dim)
NSUB = N // SUB           # 50
NROT = 25                 # distinct values of (128*i) % 200
NROTX = 28                # extended with 3 duplicates so chunks never wrap
NIDXCOL = N // 16         # idx columns (wrapped in 16 partitions)
# chunk starts i0 must satisfy (i0 % NROT) + ck <= NROTX so each chunk's
# positional block is contiguous in the rotated table; chunks are capped
# at 7 subtiles (56 descriptors per SDMA engine) so each dma_gather fits
# the Q7's per-engine descriptor ring without mid-op reclaim -- larger
# ops crash (single_packet) or race (multi-packet) on real HW
CHUNKS = [7, 7, 7, 7, 7, 7, 7, 1]

F32 = mybir.dt.float32
F16 = mybir.dt.float16
I16 = mybir.dt.int16

assert sum(CHUNKS) == NSUB
_i0 = 0
for _ck in CHUNKS:
    assert (_i0 % NROT) + _ck <= NROTX, (_i0, _ck)
    _i0 += _ck


def emit_core_kernel(tc, idx, ctab, posrot, out):
    """Emit the per-core kernel into TileContext `tc`.

    idx    : DRAM [128, NIDXCOL] int16 compact token ids, wrapped layout:
             token n at (n % 16, n // 16), replicated x8 down partitions
    ctab   : DRAM [N, D] f16 per-core compact fused table
    posrot : DRAM [128, NROTX*D] f16 rotated positional table
    out    : DRAM [128, NSUB, D] f16, out[p, i, :] = embedding of token i*128+p
    """
    nc = tc.nc

    with (
        tc.tile_pool(name="const", bufs=1) as cpool,
        tc.tile_pool(name="work", bufs=2) as wpool,
    ):
        # idx first: the first gather depends only on its first slice
        idx_sb = cpool.tile([128, NIDXCOL], I16)
        c0 = CHUNKS[0] * 8
        nc.sync.dma_start(out=idx_sb[:, 0:c0], in_=idx[:, 0:c0])
        nc.sync.dma_start(out=idx_sb[:, c0:NIDXCOL], in_=idx[:, c0:NIDXCOL])
        posrot_sb = cpool.tile([128, NROTX * D], F16)
        nc.sync.dma_start(out=posrot_sb[:], in_=posrot)

        i0 = 0
        for ck in CHUNKS:
            # one batched gather: ck*128 tokens, one 256B descriptor each;
            # token i*128+p lands on partition p, local subtile i
            g_sb = wpool.tile([128, ck * D], F16, tag="g", bufs=3)
            nc.gpsimd.dma_gather(
                out_ap=g_sb[:].rearrange("p (j d) -> p j d", d=D),
                in_ap=ctab,
                idxs_ap=idx_sb[:, i0 * 8:(i0 + ck) * 8],
                num_idxs=ck * SUB,
                num_idxs_reg=ck * SUB,
                elem_size=D,
            )
            # fused positional add, one DVE op per chunk
            r0 = i0 % NROT
            o_sb = wpool.tile([128, ck * D], F16, tag="o", bufs=3)
            nc.vector.tensor_tensor(
                out=o_sb[:],
                in0=g_sb[:],
                in1=posrot_sb[:, r0 * D:(r0 + ck) * D],
                op=mybir.AluOpType.add,
            )
            nc.sync.dma_start(
                out=out[:, i0:i0 + ck, :],
                in_=o_sb[:].rearrange("p (j d) -> p j d", d=D),
            )
            i0 += ck


def build_nc():
    nc = bacc.Bacc("TRN2", target_bir_lowering=False, debug=False)
    idx = nc.dram_tensor("idx", [128, NIDXCOL], I16, kind="ExternalInput").ap()
    ctab = nc.dram_tensor("ctab", [N, D], F16, kind="ExternalInput").ap()
    posrot = nc.dram_tensor(
        "posrot", [128, NROTX * D], F16, kind="ExternalInput").ap()
    out = nc.dram_tensor("out", [128, NSUB, D], F16, kind="ExternalOutput").ap()

    with tile.TileContext(nc) as tc:
        emit_core_kernel(tc, idx, ctab, posrot, out)
    nc.compile()
    return nc


_NC_CACHE = None


def _get_nc():
    global _NC_CACHE
    if _NC_CACHE is None:
        _NC_CACHE = build_nc()
    return _NC_CACHE


def make_ftab(token_table, genre_table, token_genre_ids, genre_counts):
    """fused table: ftab[v] = token_table[v] + mean genre row, f32."""
    gids = np.asarray(token_genre_ids).astype(np.int64)        # [V, MAXG]
    cnt = np.asarray(genre_counts).astype(np.int64)            # [V]
    mask = np.arange(MAXG)[None, :] < cnt[:, None]             # [V, MAXG]
    flat = (np.arange(VOCAB, dtype=np.int64)[:, None] * G + gids)[mask]
    W = np.bincount(flat, minlength=VOCAB * G).reshape(VOCAB, G)
    gmean = (W.astype(np.float32) @ np.asarray(genre_table, dtype=np.float32))
    gmean /= cnt[:, None].astype(np.float32)
    return np.asarray(token_table, dtype=np.float32) + gmean


def make_posrot(pos_table):
    pos = np.asarray(pos_table, dtype=np.float32)
    pr = np.zeros((128, NROTX * D), dtype=np.float32)
    p = np.arange(128)
    for r in range(NROTX):
        pr[:, r * D:(r + 1) * D] = pos[(128 * r + p) % L, :]
    return pr.astype(np.float16)


def prep_host_inputs(sequence, token_table, genre_table, pos_table,
                     token_genre_ids, genre_counts):
    """Host-side sharding / layout prep. Returns in_maps for the 8 cores."""
    seq = np.ascontiguousarray(
        np.asarray(sequence).astype(np.int64)).reshape(B, L)
    ftab = make_ftab(token_table, genre_table, token_genre_ids, genre_counts)
    posrot = make_posrot(pos_table)

    in_maps = []
    for c in range(NCORES):
        seq_core = seq[c * BC:(c + 1) * BC].reshape(N)
        uniq, inv = np.unique(seq_core, return_inverse=True)
        ctab = np.zeros((N, D), dtype=np.float16)
        ctab[:len(uniq)] = ftab[uniq].astype(np.float16)
        # wrapped idx layout: token n at (n % 16, n // 16), replicated x8
        idx16 = np.ascontiguousarray(
            inv.astype(np.int16).reshape(NIDXCOL, 16).T)
        idx = np.tile(idx16, (8, 1))
        in_maps.append({
            "idx": idx,
            "ctab": ctab,
            "posrot": posrot,
        })
    return in_maps


def postprocess(results):
    """Un-permute per-core outputs and concatenate to [B, L, D] f32."""
    outs = []
    for c in range(NCORES):
        o = results[c]["out"].astype(np.float32)  # [128, NSUB, D]
        outs.append(
            np.ascontiguousarray(o.transpose(1, 0, 2)).reshape(BC, L, D))
    return np.concatenate(outs, axis=0)


def kernel(sequence, token_table, genre_table, pos_table, token_genre_ids,
           genre_counts):
    nc = _get_nc()
    in_maps = prep_host_inputs(sequence, token_table, genre_table, pos_table,
                               token_genre_ids, genre_counts)
    res = run_bass_kernel_spmd(nc, in_maps, core_ids=list(range(NCORES)))
    return postprocess(res.results)
